# revision 3
# baseline (speedup 1.0000x reference)
"""Trainium2 Bass kernel for nn_AttentionCrossLayer.

Math: in the reference, softmax over a length-1 axis is exactly 1.0, so
attn == v and q/k/wq/wk are dead code. With x0 the (never-mutated) input,
each layer's gate xw_i = out_i @ cw_i is a fixed linear function of x0:
    xw_i = x0 @ u_i + c_i,   u_i = Wv_i @ (Wo_i @ cw_i),
                             c_i = (bv_i @ Wo_i + bo_i) @ cw_i
and the layer recurrence x += x0 * xw_i + cb_i telescopes to
    out[b, d] = x0[b, d] * (x0[b, :] @ usum + cprime) + cbsum[d]
with usum = sum_i u_i  [D], cprime = 1 + sum_i c_i, cbsum = sum_i cb_i [D].

The tiny weight contractions happen host-side in float64. The device
kernel makes one pass over x per core: a fused multiply+row-reduce
(scalar_tensor_tensor with accum_out on the Vector engine) produces the
per-row gate t, then a fused scale-and-add produces the output tile.
The cprime term rides in an extra constant column appended to x/u so the
row-reduce emits the finished gate with no fix-up op.

Sharding: data-parallel over batch across 8 cores, weights replicated,
no cross-device communication. Raw Bass (no Tile): explicit semaphores,
NBUF-deep double buffering; loads on the sync engine (HWDGE), stores on
gpsimd (SWDGE), compute on the Vector engine.
"""

import numpy as np

L, B, D, H, K = 3, 32768, 1024, 8, 64
N_CORES = 8
B_LOC = B // N_CORES  # 4096 rows per core
P = 128
N_TILES = B_LOC // P  # 32
NBUF = 4

_cache = {}


def _build_program(cprime: float):
    import concourse.bass as bass
    from concourse import mybir

    F32 = mybir.dt.float32
    MUL = mybir.AluOpType.mult
    ADD = mybir.AluOpType.add

    nc = bass.Bass()
    x = nc.declare_dram_parameter("x", [B_LOC, D], F32, isOutput=False)
    u = nc.declare_dram_parameter("u", [1, D], F32, isOutput=False)
    cb = nc.declare_dram_parameter("cb", [1, D], F32, isOutput=False)
    out = nc.declare_dram_parameter("out", [B_LOC, D], F32, isOutput=True)

    u_bcast = bass.AP(tensor=u.ap().tensor, offset=0, ap=[[0, P], [1, D]])
    cb_bcast = bass.AP(tensor=cb.ap().tensor, offset=0, ap=[[0, P], [1, D]])

    with (
        nc.sbuf_tensor([P, D + 1], F32) as ub,  # [:, :D]=usum, [:, D]=cprime
        nc.sbuf_tensor([P, D], F32) as cbb,
        nc.sbuf_tensor([P, NBUF, D + 1], F32) as xt,  # [:, s, D] = 1.0
        nc.sbuf_tensor([P, 2, D + 1], F32) as oscr,  # throwaway STT main out
        nc.sbuf_tensor([P, NBUF, D], F32) as o2,
        nc.sbuf_tensor([P, NBUF, 1], F32) as tsc,
        # One DMA outstanding per semaphore: a multi-queue DMA increments
        # its sem in fractions of 16, so cumulative prefix-waits over a
        # shared sem would fire early. Per-slot sems avoid that entirely.
        nc.semaphore("us") as us,
        nc.semaphore("ld0") as ld0,
        nc.semaphore("ld1") as ld1,
        nc.semaphore("ld2") as ld2,
        nc.semaphore("ld3") as ld3,
        nc.semaphore("st0") as st0,
        nc.semaphore("st1") as st1,
        nc.semaphore("st2") as st2,
        nc.semaphore("st3") as st3,
        nc.semaphore("cm") as cm,
        nc.semaphore("cm2") as cm2,
        nc.Block() as block,
    ):
        lds = [ld0, ld1, ld2, ld3]
        sts = [st0, st1, st2, st3]

        @block.sync
        def _(sync):
            sync.dma_start(out=ub[:, 0:D], in_=u_bcast).then_inc(us, 16)
            sync.dma_start(out=cbb[:, :], in_=cb_bcast).then_inc(us, 16)
            for i in range(N_TILES):
                s = i % NBUF
                if i >= NBUF:
                    # xt slot free once compute pass 2 of i-NBUF retired
                    sync.wait_ge(cm2, i - NBUF + 1)
                sync.dma_start(
                    out=xt[:, s, 0:D], in_=x[i * P : (i + 1) * P, :]
                ).then_inc(lds[s], 16)

        @block.vector
        def _(vector):
            vector.memset(ub[:, D : D + 1], cprime)
            for s in range(NBUF):
                vector.memset(xt[:, s, D : D + 1], 1.0)
            vector.wait_ge(us, 32)
            for i in range(N_TILES):
                s = i % NBUF
                vector.wait_ge(lds[s], 16 * (i // NBUF + 1))
                # oscr = x' * u' ; t = sum_free(oscr) = x.usum + cprime
                nc.vector.scalar_tensor_tensor(
                    out=oscr[:, i % 2, :],
                    in0=xt[:, s, :],
                    scalar=1.0,
                    in1=ub[:, :],
                    op0=MUL,
                    op1=MUL,
                    accum_out=tsc[:, s, :],
                ).then_inc(cm, 1)
                # t's accumulator writeback must retire before it is read
                vector.wait_ge(cm, i + 1)
                if i >= NBUF:
                    # o2 slot free once store of i-NBUF finished reading
                    vector.wait_ge(sts[s], 16 * (i // NBUF))
                # o2 = x * t + cbsum
                nc.vector.scalar_tensor_tensor(
                    out=o2[:, s, :],
                    in0=xt[:, s, 0:D],
                    scalar=tsc[:, s, :],
                    in1=cbb[:, :],
                    op0=MUL,
                    op1=ADD,
                ).then_inc(cm2, 1)

        @block.gpsimd
        def _(gpsimd):
            for i in range(N_TILES):
                s = i % NBUF
                gpsimd.wait_ge(cm2, i + 1)
                gpsimd.dma_start(
                    out=out[i * P : (i + 1) * P, :], in_=o2[:, s, :]
                ).then_inc(sts[s], 16)
            for s in range(NBUF):
                gpsimd.wait_ge(sts[s], 16 * (N_TILES // NBUF))

    return nc


def _precompute(wv, bv, wo, bo, cw, cb):
    """Host-side f64 contraction of the small per-layer weights."""
    usum = np.zeros(D, np.float64)
    cprime = 1.0
    for i in range(L):
        Wv = wv[i].reshape(D, H * K).astype(np.float64)
        Wo = wo[i].reshape(H * K, D).astype(np.float64)
        cwi = cw[i].reshape(D).astype(np.float64)
        wocw = Wo @ cwi
        usum += Wv @ wocw
        cprime += float(bv[i].reshape(H * K).astype(np.float64) @ wocw)
        cprime += float(bo[i].astype(np.float64) @ cwi)
    cbsum = cb.astype(np.float64).sum(axis=0)
    return usum.astype(np.float32), float(np.float32(cprime)), cbsum.astype(np.float32)


def kernel(x, wq, bq, wk, bk, wv, bv, wo, bo, cw, cb):
    from concourse.bass_utils import run_bass_kernel_spmd

    x = np.ascontiguousarray(np.asarray(x, dtype=np.float32))
    usum, cprime, cbsum = _precompute(
        np.asarray(wv), np.asarray(bv), np.asarray(wo), np.asarray(bo),
        np.asarray(cw), np.asarray(cb),
    )

    if cprime not in _cache:
        _cache[cprime] = _build_program(cprime)
    nc = _cache[cprime]

    u2 = usum.reshape(1, D)
    cb2 = cbsum.reshape(1, D)
    in_maps = [
        {"x": x[c * B_LOC : (c + 1) * B_LOC], "u": u2, "cb": cb2}
        for c in range(N_CORES)
    ]
    res = run_bass_kernel_spmd(nc, in_maps, list(range(N_CORES)))
    return np.concatenate([res.results[c]["out"] for c in range(N_CORES)], axis=0)
